# revision 1
# baseline (speedup 1.0000x reference)
"""GCN 2-layer encoder on 8 Trainium2 NeuronCores — zero-collective design.

Every core redundantly computes the cheap dense transforms for ALL nodes
(y1 = dinv*(x@W1), zT = relu-agg, y2 = dinv*(z@W2)) into LOCAL bf16 HBM
gather tables, so no AllGather is ever needed (collectives cost ~5ms each
on this runtime when awaited).  Layer-1 aggregation is replicated over all
100352 padded nodes (one-hot matmul segment-sum, gather-bandwidth bound);
layer-2 aggregation only covers the core's own 12500 dst nodes.

Messages are bf16 (halves gather bytes; PSUM accumulates f32).  The
one-hot S_t fuses dinv[dst] via tensor_scalar(is_equal, mult) with
per-op dstmS / per-chunk dd streamed from HBM per window-block.
"""
import os
import sys

sys.path.insert(0, "/opt/trn_rl_repo")
import numpy as np
import ml_dtypes

import concourse.tile as tile
from concourse import bacc, mybir, library_config
from concourse.bass_utils import run_bass_kernel_spmd

N_NODES = 100000
N_CORES = 8
S = N_NODES // N_CORES          # 12500 own nodes per core
D = 128
NPAD = 100352                   # 784 * 128
NWG = NPAD // 128               # 784 global dst windows (layer 1)
WPT = 12                        # windows per block (3 PSUM banks x 4)
NWBG = (NWG + WPT - 1) // WPT   # 66 global window blocks
QL = 25088                      # quarter length (196*128), int16-safe
QBG = [0, QL, 2 * QL, 3 * QL, 4 * QL]
NW2 = (S + 127) // 128          # 98 own dst windows (layer 2)
NWB2 = (NW2 + WPT - 1) // WPT   # 9
CALL = int(os.environ.get("KERNEL_CALL", "1024"))
f32 = mybir.dt.float32
bf16 = mybir.dt.bfloat16
i16 = mybir.dt.int16


def _pad128(n):
    return max(128, ((n + 127) // 128) * 128)


def _wrap_idx(gidx_flat):
    """[slots] int16 -> [128, slots/16] wrapped+replicated for dma_gather."""
    a = gidx_flat.reshape(-1, 16).T
    return np.tile(a, (8, 1)).copy()


def _sched_edges(src, dst, dinv, nwb, n_seg_q, dst_base=0, with_dd=True):
    """Common scheduling: segment edges by (window-block of dst, quarter of
    src), pad each segment to x128, emit chunk/op tables.

    dst here is already local (dst_base subtracted).  Returns dict with
    gidx [slots], dstm [128, nch] (local dst value per slot), per-op arrays,
    and op metadata list (b, q, k, jc, jl, wl, start, stop, oi).
    """
    w = dst // 128
    b = w // WPT
    q = src // QL
    if os.environ.get("SORT_DST"):
        order = np.lexsort((src, dst, q, b))
    else:
        order = np.lexsort((src, w, q, b))
    so, do, qo, bo = src[order], dst[order], q[order], b[order]
    segkey = bo * 4 + qo
    n = len(so)
    bounds = np.flatnonzero(np.diff(segkey)) + 1
    starts = np.concatenate([[0], bounds])
    ends = np.concatenate([bounds, [n]])
    seg_edges = {int(segkey[s0]): (s0, e0) for s0, e0 in zip(starts, ends)}

    seg_list = [(bb, qq) for bb in range(nwb) for qq in range(4)]
    L = {}
    for (bb, qq) in seg_list:
        k = bb * 4 + qq
        ln = seg_edges[k][1] - seg_edges[k][0] if k in seg_edges else 0
        L[(bb, qq)] = _pad128(ln) if ln else 0
    tot = sum(L.values())
    gidx = np.zeros(tot, np.int16)
    dstv = np.full(tot, -1.0e6, np.float64)
    ddv = np.zeros(tot, np.float64)
    seg_base = {}
    off = 0
    for (bb, qq) in seg_list:
        seg_base[(bb, qq)] = off
        k = bb * 4 + qq
        if k in seg_edges:
            s0, e0 = seg_edges[k]
            m = e0 - s0
            gidx[off:off + m] = (so[s0:e0] - QBG[qo[s0]]).astype(np.int16)
            dstv[off:off + m] = do[s0:e0]
            if with_dd:
                ddv[off:off + m] = dinv[do[s0:e0] + dst_base]
        off += L[(bb, qq)]

    nch = tot // 128
    slot_ch = np.arange(tot) // 128
    real = dstv >= 0
    wl_slot = np.where(real, (dstv // 128).astype(np.int64), 0)
    # ops: unique (chunk, window) among real slots
    opkey = slot_ch * NWG + wl_slot
    op_keys = np.unique(opkey[real])
    n_ops = len(op_keys)
    op_of_slot = np.searchsorted(op_keys, opkey)
    # per-op dstm (dst - 128*w for slots in that op's window, else -1000)
    dstmS = np.full((n_ops, 128), -1000.0, np.float32)
    sl = np.flatnonzero(real)
    dstmS[op_of_slot[sl], sl % 128] = (dstv[sl] - 128.0 * wl_slot[sl])
    dstmS = np.ascontiguousarray(dstmS.T)             # [128, n_ops]
    ddT = np.zeros((nch, 128), np.float32)
    ddT[slot_ch[sl], sl % 128] = ddv[sl]
    ddT = np.ascontiguousarray(ddT.T)                 # [128, nch]
    dstmC = np.full((nch, 128), -1.0e6, np.float32)
    dstmC[slot_ch[sl], sl % 128] = dstv[sl]
    dstmC = np.ascontiguousarray(dstmC.T)             # [128, nch]

    # op metadata in emission order (seg-major)
    op_ch = op_keys // NWG
    op_w = op_keys % NWG
    ops = []
    first_bk, last_bk = {}, {}
    for oi in range(n_ops):
        j = int(op_ch[oi])
        wv = int(op_w[oi])
        base = j * 128
        # find seg of this chunk
        bb = int(wv // WPT)
        qq = None
        for qx in range(4):
            sb = seg_base[(bb, qx)]
            if sb <= base < sb + L[(bb, qx)]:
                qq = qx
                break
        assert qq is not None, (j, wv, bb)
        jseg = (base - seg_base[(bb, qq)]) // 128
        ops.append([bb, qq, jseg, j, wv, oi, False, False])
        bk = (bb, (wv - bb * WPT) // 4)
        if bk not in first_bk:
            first_bk[bk] = len(ops) - 1
        last_bk[bk] = len(ops) - 1
    for i in first_bk.values():
        ops[i][6] = True
    for i in last_bk.values():
        ops[i][7] = True
    ops_by_seg = {sk: [] for sk in seg_list}
    for op in ops:
        ops_by_seg[(op[0], op[1])].append(op)
    # interleave ops across windows within each segment so consecutive PE
    # matmuls hit different PSUM banks/addresses (avoids accumulation-drain
    # serialization on same-window chunk runs)
    if not os.environ.get("NO_INTERLEAVE"):
        for sk in seg_list:
            lst = ops_by_seg[sk]
            cnt = {}
            keyed = []
            for op in lst:
                r = cnt.get(op[4], 0)
                cnt[op[4]] = r + 1
                wl = op[4] - op[0] * WPT
                keyed.append((r, wl % 4, wl // 4, op))
            lst2 = [t[3] for t in sorted(keyed,
                                         key=lambda t: (t[0], t[1], t[2]))]
            ops_by_seg[sk] = lst2
        # recompute start/stop flags in final emission order
        flat = []
        for sk in seg_list:
            flat.extend(ops_by_seg[sk])
        for op in flat:
            op[6] = op[7] = False
        first_bk, last_bk = {}, {}
        for i, op in enumerate(flat):
            bk = (op[0], (op[4] - op[0] * WPT) // 4)
            if bk not in first_bk:
                first_bk[bk] = i
            last_bk[bk] = i
        for i in first_bk.values():
            flat[i][6] = True
        for i in last_bk.values():
            flat[i][7] = True
    touched_w = set(int(x) for x in np.unique(op_w))
    return {
        "L": L, "seg_base": seg_base, "seg_list": seg_list,
        "tot": tot, "nch": nch, "n_ops": n_ops,
        "gidx": gidx, "dstmS": dstmS, "ddT": ddT, "dstmC": dstmC,
        "ops_by_seg": ops_by_seg, "touched_w": touched_w,
    }


def _prep(edge_index):
    """Returns (sched_l1, scheds_l2, dinv). sched_l1 is common to all
    cores; scheds_l2 is per-core with common shapes/op-structure."""
    src = np.asarray(edge_index[0], dtype=np.int64)
    dst = np.asarray(edge_index[1], dtype=np.int64)
    deg = (np.bincount(dst, minlength=N_NODES) + 1).astype(np.float64)
    dinv = (1.0 / np.sqrt(deg)).astype(np.float64)

    loop = np.arange(N_NODES, dtype=np.int64)
    src_all = np.concatenate([src, loop])
    dst_all = np.concatenate([dst, loop])

    s1 = _sched_edges(src_all, dst_all, dinv, NWBG, 4)

    # layer 2: per-core own dst, common padding/op structure
    core = dst_all // S
    percore = []
    for c in range(N_CORES):
        m = core == c
        percore.append(_sched_edges(src_all[m], dst_all[m] - c * S, dinv,
                                    NWB2, 4, dst_base=c * S, with_dd=False))
    # unify L across cores, rebuild with common layout
    Lc = {}
    for sk in percore[0]["L"]:
        Lc[sk] = max(p["L"][sk] for p in percore)
    totc = sum(Lc.values())
    nchc = totc // 128
    seg_base = {}
    off = 0
    for sk in percore[0]["seg_list"]:
        seg_base[sk] = off
        off += Lc[sk]
    # re-embed each core's slot arrays into the common layout
    gidx2 = np.zeros((N_CORES, totc), np.int16)
    dstm2 = np.full((N_CORES, 128, nchc), -1.0e6, np.float32)
    opset = set()
    for c in range(N_CORES):
        p = percore[c]
        for sk in p["seg_list"]:
            sb_c, sb_u = p["seg_base"][sk], seg_base[sk]
            ln = p["L"][sk]
            if ln == 0:
                continue
            gidx2[c, sb_u:sb_u + ln] = p["gidx"][sb_c:sb_c + ln]
            for j in range(ln // 128):
                dstm2[c, :, (sb_u // 128) + j] = \
                    p["dstmC"][:, (sb_c // 128) + j]
        # collect op (chunk_global_unified, window) pairs
        for sk in p["seg_list"]:
            for op in p["ops_by_seg"][sk]:
                bb, qq, jseg, _, wv = op[0], op[1], op[2], op[3], op[4]
                ju = (seg_base[(bb, qq)] // 128) + jseg
                opset.add((bb, qq, jseg, ju, wv))
    ops2 = sorted(opset, key=lambda t: (t[0], t[1], t[2], t[4]))
    ops_list = []
    first_bk, last_bk = {}, {}
    for (bb, qq, jseg, ju, wv) in ops2:
        ops_list.append([bb, qq, jseg, ju, wv, None, False, False])
        bk = (bb, (wv - bb * WPT) // 4)
        if bk not in first_bk:
            first_bk[bk] = len(ops_list) - 1
        last_bk[bk] = len(ops_list) - 1
    for i in first_bk.values():
        ops_list[i][6] = True
    for i in last_bk.values():
        ops_list[i][7] = True
    ops_by_seg = {sk: [] for sk in percore[0]["seg_list"]}
    for op in ops_list:
        ops_by_seg[(op[0], op[1])].append(op)
    touched = set()
    for c in range(N_CORES):
        touched |= percore[c]["touched_w"]
    sched_l2 = {
        "L": Lc, "seg_base": seg_base, "tot": totc, "nch": nchc,
        "seg_list": percore[0]["seg_list"],
        "gidx": gidx2, "dstm": dstm2, "ops_by_seg": ops_by_seg,
        "touched_w": touched,
    }
    return s1, sched_l2, dinv.astype(np.float32)


def _build(s1, s2, repeat=1, skip_mm=False, contig_gather=False,
           skip_dve=False, ohb=4):
    nc = bacc.Bacc("TRN2", target_bir_lowering=False, debug=False,
                   num_devices=N_CORES, num_swdge_queues=4)
    xT = nc.dram_tensor("xT", [128, NPAD], f32, kind="ExternalInput")
    W1 = nc.dram_tensor("W1", [128, 128], f32, kind="ExternalInput")
    W2b = nc.dram_tensor("W2b", [128, 128], bf16, kind="ExternalInput")
    b1c = nc.dram_tensor("b1c", [128, 1], f32, kind="ExternalInput")
    b2b = nc.dram_tensor("b2b", [128, 128], f32, kind="ExternalInput")
    dinvg = nc.dram_tensor("dinvg", [128, NWG], f32, kind="ExternalInput")
    dinv2 = nc.dram_tensor("dinv2", [128, NW2], f32, kind="ExternalInput")
    iotab = nc.dram_tensor("iotab", [128, 128], bf16, kind="ExternalInput")
    gx1 = nc.dram_tensor("gx1", [128, s1["tot"] // 16], i16,
                         kind="ExternalInput")
    dS1 = nc.dram_tensor("dS1", [128, s1["n_ops"]], f32,
                         kind="ExternalInput")
    dd1 = nc.dram_tensor("dd1", [128, s1["nch"]], f32, kind="ExternalInput")
    gx2 = nc.dram_tensor("gx2", [128, s2["tot"] // 16], i16,
                         kind="ExternalInput")
    dm2 = nc.dram_tensor("dm2", [128, s2["nch"]], f32, kind="ExternalInput")
    out = nc.dram_tensor("out", [S, D], f32, kind="ExternalOutput")

    y1t = [nc.dram_tensor(f"y1t{q}", [QL, 128], bf16) for q in range(4)]
    y2t = [nc.dram_tensor(f"y2t{q}", [QL, 128], bf16) for q in range(4)]
    zT = nc.dram_tensor("zT", [128, NPAD], bf16)

    # per-wb column ranges of dS1/dd1 for streaming
    op_base1, ch_base1 = [], []
    nop_acc = 0
    for bb in range(NWBG):
        op_base1.append(nop_acc)
        nop_acc += sum(len(s1["ops_by_seg"][(bb, qq)]) for qq in range(4))
    op_base1.append(nop_acc)
    assert nop_acc == s1["n_ops"]
    for bb in range(NWBG):
        ch_base1.append(s1["seg_base"][(bb, 0)] // 128)
    ch_base1.append(s1["nch"])

    wpwb_g = [min(NWG - bb * WPT, WPT) for bb in range(NWBG)]
    wpwb_2 = [min(NW2 - bb * WPT, WPT) for bb in range(NWB2)]

    with tile.TileContext(nc) as tc:
        with (
            tc.tile_pool(name="cst", bufs=1) as cst,
            tc.tile_pool(name="blk", bufs=4) as blkp,
            tc.tile_pool(name="ps", bufs=2, space="PSUM") as php,
            tc.tile_pool(name="st", bufs=8) as stp,
            tc.tile_pool(name="gxp", bufs=3) as gxp,
            tc.tile_pool(name="dsp", bufs=2) as dsp,
            tc.tile_pool(name="oh", bufs=ohb) as ohp,
            tc.tile_pool(name="bank", bufs=1, space="PSUM") as bankp,
            tc.tile_pool(name="fl", bufs=4) as flp,
        ):
            nc.gpsimd.load_library(library_config.mlp)

            W1_sb = cst.tile([128, 128], f32, tag="W1")
            W2_sb = cst.tile([128, 128], bf16, tag="W2")
            b1_sb = cst.tile([128, 1], f32, tag="b1")
            b2_sb = cst.tile([128, 128], f32, tag="b2")
            dinvg_sb = cst.tile([128, NWG], f32, tag="dg")
            dinv2_sb = cst.tile([128, NW2], f32, tag="d2")
            iota_sb = cst.tile([128, 128], bf16, tag="iota")
            dm2_sb = cst.tile([128, s2["nch"]], f32, tag="dm2")
            nc.sync.dma_start(W1_sb[:], W1[:])
            nc.sync.dma_start(W2_sb[:], W2b[:])
            nc.sync.dma_start(b1_sb[:], b1c[:])
            nc.sync.dma_start(b2_sb[:], b2b[:])
            nc.sync.dma_start(dinvg_sb[:], dinvg[:])
            nc.sync.dma_start(dinv2_sb[:], dinv2[:])
            nc.sync.dma_start(iota_sb[:], iotab[:])
            nc.sync.dma_start(dm2_sb[:], dm2[:])
            banks = [bankp.tile([128, 512], f32, tag=f"bk{i}",
                                name=f"bank{i}") for i in range(6)]

            loop_cm = tc.For_i(0, repeat, 1) if repeat > 1 else None
            if loop_cm is not None:
                loop_cm.__enter__()

            # ---- phase A: y1 tables (all nodes) ----
            def build_table(src_dram, w_sb, tabs, src_dt):
                G = 8
                for g0 in range(0, NWG, G):
                    ng = min(G, NWG - g0)
                    xt = blkp.tile([128, G * 128], src_dt, tag="xt")
                    nc.sync.dma_start(
                        xt[:, :ng * 128],
                        src_dram.ap()[:, 128 * g0:128 * (g0 + ng)])
                    yb = blkp.tile([128, G * 128], bf16, tag="yb")
                    for i in range(ng):
                        t = g0 + i
                        ps = php.tile([128, 128], f32, tag="php")
                        nc.tensor.matmul(
                            ps[:], lhsT=xt[:, 128 * i:128 * (i + 1)],
                            rhs=w_sb[:], start=True, stop=True)
                        nc.vector.tensor_scalar(
                            yb[:, 128 * i:128 * (i + 1)], ps[:],
                            dinvg_sb[:, t:t + 1], None,
                            op0=mybir.AluOpType.mult)
                    # DMA out, split at quarter boundaries
                    i = 0
                    while i < ng:
                        t = g0 + i
                        q = (128 * t) // QL
                        nblk = min(ng - i, (QBG[q + 1] - 128 * t) // 128)
                        r0 = 128 * t - QBG[q]
                        nc.sync.dma_start(
                            tabs[q].ap()[r0:r0 + 128 * nblk, :].rearrange(
                                "(t p) f -> p t f", p=128),
                            yb[:, 128 * i:128 * (i + nblk)].rearrange(
                                "p (t f) -> p t f", f=128))
                        i += nblk

            build_table(xT, W1_sb, y1t, f32)

            # ---- phase B: layer-1 aggregation over all nodes -> zT ----
            def aggregate(sched, tabs, gx_dram, nwb, wpwb, l2=False,
                          dS_dram=None, dd_dram=None, opb=None, chb=None):
                qctr = [0]
                for bb in range(nwb):
                    wb_base = sched["seg_base"][(bb, 0)]
                    wb_len = sum(sched["L"][(bb, qx)] for qx in range(4))
                    gx_t = gxp.tile([128, (wb_len + 15) // 16], i16,
                                    tag="gx")
                    if wb_len:
                        nc.sync.dma_start(
                            gx_t[:],
                            gx_dram.ap()[:, wb_base // 16:
                                         (wb_base + wb_len) // 16])
                    if not l2:
                        no = opb[bb + 1] - opb[bb]
                        nch_b = chb[bb + 1] - chb[bb]
                        dS_t = dsp.tile([128, max(no, 1)], f32, tag="dS",
                                        name=f"dS{bb}")
                        if no:
                            nc.sync.dma_start(
                                dS_t[:], dS_dram.ap()[:, opb[bb]:opb[bb + 1]])
                        dd_t = dsp.tile([128, max(nch_b, 1)], f32, tag="dd",
                                        name=f"dd{bb}")
                        if nch_b:
                            nc.sync.dma_start(
                                dd_t[:], dd_dram.ap()[:, chb[bb]:chb[bb + 1]])
                    for qq in range(4):
                        Lseg = sched["L"][(bb, qq)]
                        if Lseg == 0:
                            continue
                        base = sched["seg_base"][(bb, qq)]
                        sb_l = base - wb_base
                        ncalls = (Lseg + CALL - 1) // CALL
                        stages = []
                        for k in range(ncalls):
                            cl = min(CALL, Lseg - CALL * k)
                            stg = stp.tile([128, CALL // 128, 128], bf16,
                                           tag="stg")
                            if contig_gather:
                                r0 = (base + CALL * k) % (QL - CALL)
                                nc.sync.dma_start(
                                    stg[:, :cl // 128, :],
                                    tabs[qq].ap()[r0:r0 + cl, :].rearrange(
                                        "(t p) f -> p t f", p=128))
                            else:
                                nc.gpsimd.dma_gather(
                                    stg[:, :cl // 128, :], tabs[qq].ap(),
                                    gx_t[:, (sb_l + CALL * k) // 16:
                                         (sb_l + CALL * k + cl) // 16],
                                    cl, cl, 128,
                                    queue_num=qctr[0] % 4)
                            qctr[0] += 1
                            stages.append(stg)
                        for op in sched["ops_by_seg"][(bb, qq)]:
                            _, _, jseg, jg, wv, oi, st_f, sp_f = op
                            k, jc = jseg // (CALL // 128), \
                                jseg % (CALL // 128)
                            wl = wv - bb * WPT
                            bank = banks[(bb % 2) * 3 + wl // 4]
                            bsl = bank[:, 128 * (wl % 4):128 * (wl % 4 + 1)]
                            S_t = ohp.tile([128, 128], bf16, tag="S")
                            if skip_mm:
                                continue
                            if l2:
                                nc.vector.tensor_scalar(
                                    S_t[:], iota_sb[:],
                                    dm2_sb[:, jg:jg + 1], float(-128.0 * wv),
                                    op0=mybir.AluOpType.subtract,
                                    op1=mybir.AluOpType.is_equal)
                                nc.tensor.matmul(
                                    bsl, lhsT=S_t[:], rhs=stages[k][:, jc, :],
                                    start=st_f, stop=sp_f)
                            else:
                                oloc = oi - opb[bb]
                                jloc = jg - chb[bb]
                                nc.vector.tensor_scalar(
                                    S_t[:], iota_sb[:],
                                    dS_t[:, oloc:oloc + 1],
                                    dd_t[:, jloc:jloc + 1],
                                    op0=mybir.AluOpType.is_equal,
                                    op1=mybir.AluOpType.mult)
                                nc.tensor.matmul(
                                    bsl, lhsT=stages[k][:, jc, :], rhs=S_t[:],
                                    start=st_f, stop=sp_f)
                    # flush (batched per block)
                    nwin = wpwb[bb]
                    if l2:
                        ob = flp.tile([128, WPT * 128], f32, tag="ow")
                    else:
                        ob = flp.tile([128, WPT * 128], bf16, tag="zw")
                    for wl in range(nwin):
                        wv = bb * WPT + wl
                        bank = banks[(bb % 2) * 3 + wl // 4]
                        bsl = bank[:, 128 * (wl % 4):128 * (wl % 4 + 1)]
                        osl = ob[:, 128 * wl:128 * (wl + 1)]
                        if l2:
                            if skip_mm:
                                nc.vector.tensor_copy(osl, b2_sb[:])
                            elif wv in sched["touched_w"]:
                                nc.vector.tensor_scalar(
                                    osl, bsl, dinv2_sb[:, wv:wv + 1], None,
                                    op0=mybir.AluOpType.mult)
                                nc.vector.tensor_tensor(
                                    osl, osl, b2_sb[:],
                                    op=mybir.AluOpType.add)
                            else:
                                nc.vector.tensor_copy(osl, b2_sb[:])
                        else:
                            if skip_mm:
                                nc.vector.memset(osl, 0.0)
                            elif wv in sched["touched_w"]:
                                nc.vector.tensor_scalar(
                                    osl, bsl, b1_sb[:, 0:1], 0.0,
                                    op0=mybir.AluOpType.add,
                                    op1=mybir.AluOpType.max)
                            else:
                                nc.vector.memset(osl, 0.0)
                    w0 = bb * WPT
                    if l2:
                        rows = min(S, 128 * (w0 + nwin)) - 128 * w0
                        nfull = rows // 128
                        if nfull:
                            nc.sync.dma_start(
                                out.ap()[128 * w0:128 * (w0 + nfull), :]
                                .rearrange("(t p) f -> p t f", p=128),
                                ob[:, :128 * nfull].rearrange(
                                    "p (t f) -> p t f", f=128))
                        rem = rows - 128 * nfull
                        if rem:
                            nc.sync.dma_start(
                                out.ap()[128 * (w0 + nfull):
                                         128 * (w0 + nfull) + rem, :],
                                ob[0:rem, 128 * nfull:128 * (nfull + 1)])
                    else:
                        nc.sync.dma_start(
                            zT.ap()[:, 128 * w0:128 * (w0 + nwin)],
                            ob[:, :128 * nwin])

            aggregate(s1, y1t, gx1, NWBG, wpwb_g, l2=False,
                      dS_dram=dS1, dd_dram=dd1, opb=op_base1, chb=ch_base1)

            # ---- phase C: y2 tables ----
            build_table(zT, W2_sb, y2t, bf16)

            # ---- phase D: layer-2 aggregation (own nodes) -> out ----
            aggregate(s2, y2t, gx2, NWB2, wpwb_2, l2=True)

            if loop_cm is not None:
                loop_cm.__exit__(None, None, None)

    nc.compile()
    return nc


def _make_in_maps(x, W1, b1, W2, b2, s1, s2, dinv):
    xT = np.zeros((128, NPAD), np.float32)
    xT[:, :N_NODES] = np.asarray(x, np.float32).T
    iota = np.broadcast_to(np.arange(128, dtype=np.float32),
                           (128, 128)).astype(np.float32)
    dinv_pad = np.zeros(NPAD, np.float32)
    dinv_pad[:N_NODES] = dinv
    dinvg = np.ascontiguousarray(dinv_pad.reshape(NWG, 128).T)
    gx1w = _wrap_idx(s1["gidx"])
    common = {
        "xT": xT, "W1": np.asarray(W1, np.float32),
        "W2b": np.asarray(W2, np.float32).astype(ml_dtypes.bfloat16),
        "b1c": np.asarray(b1, np.float32).reshape(128, 1),
        "b2b": np.broadcast_to(np.asarray(b2, np.float32),
                               (128, 128)).copy(),
        "dinvg": dinvg, "iotab": iota.astype(ml_dtypes.bfloat16),
        "gx1": gx1w, "dS1": s1["dstmS"], "dd1": s1["ddT"],
    }
    in_maps = []
    for c in range(N_CORES):
        d2 = np.zeros(NW2 * 128, np.float32)
        d2[:S] = dinv[S * c:S * (c + 1)]
        dinv2 = np.ascontiguousarray(d2.reshape(NW2, 128).T)
        in_maps.append({
            **common,
            "dinv2": dinv2,
            "gx2": _wrap_idx(s2["gidx"][c]),
            "dm2": s2["dstm"][c],
        })
    return in_maps


def kernel(x, edge_index, W1, b1, W2, b2):
    s1, s2, dinv = _prep(edge_index)
    nc = _build(s1, s2, repeat=int(os.environ.get("KERNEL_REPEAT", "1")))
    in_maps = _make_in_maps(x, W1, b1, W2, b2, s1, s2, dinv)
    res = run_bass_kernel_spmd(nc, in_maps, core_ids=list(range(N_CORES)))
    return np.concatenate([res.results[c]["out"] for c in range(N_CORES)], 0)



# revision 12
# speedup vs baseline: 2.4001x; 2.4001x over previous
"""GCN 2-layer encoder on 8 Trainium2 NeuronCores — sharded design.

Key identity: segment_sum(norm * (x@W)[src]) == segment_sum(norm * x[src]) @ W,
so each layer gathers RAW features per edge (full per-edge norm fused into the
one-hot scatter matrix), aggregates into [own_nodes, 128], and applies the
dense 128x128 transform AFTER aggregation (8x less dense work, no gather-table
build for layer 1 at all: layer 1 gathers x directly as f32 512B rows = full
DMA descriptor rate).

Each core aggregates only its OWN 12500 dst nodes for both layers (~215k edges
per core instead of 1.7M). Between the layers one AllGather shares the z
activations (25.6MB bf16, ~250us measured on this runtime). Both layers reuse
the SAME edge schedule / gather indices / norm tables (identical edge set).

Node relabeling: new_id(g) = (g//12500)*12544 + g%12500 so each core's slice
is window-aligned (98 windows of 128). Gather tables are addressed per
quarter (25088 rows) to satisfy dma_gather's int16 index requirement.

Timing note: collectives cannot sit inside a hardware For_i loop, so the
repeat loop is Python-unrolled.
"""
import os
import sys

sys.path.insert(0, "/opt/trn_rl_repo")
import numpy as np
import ml_dtypes

import concourse.tile as tile
from concourse import bacc, mybir, library_config
from concourse.bass_utils import run_bass_kernel_spmd

N_NODES = 100000
N_CORES = 8
S = N_NODES // N_CORES          # 12500 own nodes per core
SP = 12544                      # padded own slice (98 windows x 128)
D = 128
NPAD = SP * N_CORES             # 100352 relabeled table rows
QL = 25088                      # quarter length (int16-safe)
NW = SP // 128                  # 98 own dst windows
WPT = 12                        # windows per block (3 PSUM banks x 4)
NWB = (NW + WPT - 1) // WPT     # 9 blocks
CALL = int(os.environ.get("KERNEL_CALL", "1024"))
f32 = mybir.dt.float32
bf16 = mybir.dt.bfloat16
i16 = mybir.dt.int16


def _pad128(n):
    return max(128, ((n + 127) // 128) * 128)


def _wrap_idx(gidx_flat):
    """[slots] int16 -> [128, slots/16] wrapped+replicated for dma_gather."""
    a = gidx_flat.reshape(-1, 16).T
    return np.tile(a, (8, 1)).copy()


def _sched_core(src_n, dst_l, w_norm):
    """Schedule one core's edges: segment by (block of dst window, quarter of
    src), pad each segment to x128, return slot arrays + op list.

    src_n: relabeled global src ids; dst_l: local dst (0..12499);
    w_norm: per-edge norm (f64).
    """
    w = dst_l // 128
    b = w // WPT
    q = src_n // QL
    order = np.lexsort((src_n, w, q, b))
    so, do, qo, bo = src_n[order], dst_l[order], q[order], b[order]
    no = w_norm[order]
    segkey = bo * 4 + qo
    n = len(so)
    bounds = np.flatnonzero(np.diff(segkey)) + 1
    starts = np.concatenate([[0], bounds])
    ends = np.concatenate([bounds, [n]])
    seg_edges = {int(segkey[s0]): (s0, e0) for s0, e0 in zip(starts, ends)}

    seg_list = [(bb, qq) for bb in range(NWB) for qq in range(4)]
    L = {}
    for (bb, qq) in seg_list:
        k = bb * 4 + qq
        ln = seg_edges[k][1] - seg_edges[k][0] if k in seg_edges else 0
        L[(bb, qq)] = _pad128(ln) if ln else 0
    tot = sum(L.values())
    gidx = np.zeros(tot, np.int16)
    dstv = np.full(tot, -1.0e6, np.float64)
    ddv = np.zeros(tot, np.float64)
    seg_base = {}
    off = 0
    for (bb, qq) in seg_list:
        seg_base[(bb, qq)] = off
        k = bb * 4 + qq
        if k in seg_edges:
            s0, e0 = seg_edges[k]
            m = e0 - s0
            gidx[off:off + m] = (so[s0:e0] - QL * qo[s0]).astype(np.int16)
            dstv[off:off + m] = do[s0:e0]
            ddv[off:off + m] = no[s0:e0]
        off += L[(bb, qq)]
    return {"L": L, "seg_base": seg_base, "tot": tot,
            "gidx": gidx, "dstv": dstv, "ddv": ddv}


def _prep(edge_index):
    """Returns unified per-core schedule dict."""
    src = np.asarray(edge_index[0], dtype=np.int64)
    dst = np.asarray(edge_index[1], dtype=np.int64)
    deg = (np.bincount(dst, minlength=N_NODES) + 1).astype(np.float64)
    dinv = 1.0 / np.sqrt(deg)

    loop = np.arange(N_NODES, dtype=np.int64)
    src_all = np.concatenate([src, loop])
    dst_all = np.concatenate([dst, loop])
    norm_all = dinv[src_all] * dinv[dst_all]
    src_new = (src_all // S) * SP + (src_all % S)

    core = dst_all // S
    percore = []
    for c in range(N_CORES):
        m = core == c
        percore.append(_sched_core(src_new[m], dst_all[m] - c * S,
                                   norm_all[m]))

    # unify segment lengths across cores
    seg_list = [(bb, qq) for bb in range(NWB) for qq in range(4)]
    Lc = {sk: max(p["L"][sk] for p in percore) for sk in seg_list}
    tot = sum(Lc.values())
    nch = tot // 128
    seg_base = {}
    off = 0
    for sk in seg_list:
        seg_base[sk] = off
        off += Lc[sk]
    gidx = np.zeros((N_CORES, tot), np.int16)
    dstv = np.full((N_CORES, tot), -1.0e6, np.float64)
    ddv = np.zeros((N_CORES, tot), np.float64)
    for c in range(N_CORES):
        p = percore[c]
        for sk in seg_list:
            ln = p["L"][sk]
            if ln == 0:
                continue
            sb_c, sb_u = p["seg_base"][sk], seg_base[sk]
            gidx[c, sb_u:sb_u + ln] = p["gidx"][sb_c:sb_c + ln]
            dstv[c, sb_u:sb_u + ln] = p["dstv"][sb_c:sb_c + ln]
            ddv[c, sb_u:sb_u + ln] = p["ddv"][sb_c:sb_c + ln]

    # ops: union over cores of (chunk, window) pairs with any real slot
    slot_ch = np.arange(tot) // 128
    real = dstv >= 0
    wl_slot = np.where(real, (dstv // 128).astype(np.int64), 0)
    opkey = slot_ch[None, :] * NW + wl_slot
    op_keys = np.unique(opkey[real])
    n_ops = len(op_keys)
    op_of = np.searchsorted(op_keys, opkey)
    # per-op dS (dst - 128*w for slots in that op's window, else -1000)
    dS = np.full((N_CORES, n_ops, 128), -1000.0, np.float32)
    for c in range(N_CORES):
        sl = np.flatnonzero(real[c])
        dS[c, op_of[c, sl], sl % 128] = \
            dstv[c, sl] - 128.0 * wl_slot[c, sl]
    dS = np.ascontiguousarray(dS.transpose(0, 2, 1))      # [C,128,n_ops]
    ddT = np.zeros((N_CORES, nch, 128), np.float32)
    for c in range(N_CORES):
        sl = np.flatnonzero(real[c])
        ddT[c, slot_ch[sl], sl % 128] = ddv[c, sl]
    ddT = np.ascontiguousarray(ddT.transpose(0, 2, 1))    # [C,128,nch]

    # op metadata in emission order (seg-major), interleaved across windows
    op_ch = op_keys // NW
    op_w = op_keys % NW
    ops_by_seg = {sk: [] for sk in seg_list}
    for oi in range(n_ops):
        j = int(op_ch[oi])
        wv = int(op_w[oi])
        base = j * 128
        bb = wv // WPT
        qq = None
        for qx in range(4):
            sb = seg_base[(bb, qx)]
            if sb <= base < sb + Lc[(bb, qx)]:
                qq = qx
                break
        assert qq is not None, (j, wv, bb)
        jseg = (base - seg_base[(bb, qq)]) // 128
        ops_by_seg[(bb, qq)].append([bb, qq, jseg, j, wv, oi, False, False])
    # interleave ops across windows within each segment (PSUM bank spread)
    for sk in seg_list:
        lst = ops_by_seg[sk]
        cnt = {}
        keyed = []
        for op in lst:
            r = cnt.get(op[4], 0)
            cnt[op[4]] = r + 1
            wl = op[4] - op[0] * WPT
            keyed.append((r, wl % 4, wl // 4, op))
        ops_by_seg[sk] = [t[3] for t in
                          sorted(keyed, key=lambda t: (t[0], t[1], t[2]))]
    # start/stop flags per (block, bank) in final emission order
    flat = []
    for sk in seg_list:
        flat.extend(ops_by_seg[sk])
    first_bk, last_bk = {}, {}
    for i, op in enumerate(flat):
        bk = (op[0], (op[4] - op[0] * WPT) // 4)
        if bk not in first_bk:
            first_bk[bk] = i
        last_bk[bk] = i
    for i in first_bk.values():
        flat[i][6] = True
    for i in last_bk.values():
        flat[i][7] = True
    return {
        "L": Lc, "seg_base": seg_base, "seg_list": seg_list,
        "tot": tot, "nch": nch, "n_ops": n_ops,
        "gidx": gidx, "dS": dS, "dd": ddT, "ops_by_seg": ops_by_seg,
    }


def _build(sch, repeat=1, phases="BD", ag=True, shared=True):
    nc = bacc.Bacc("TRN2", target_bir_lowering=False, debug=False,
                   num_devices=N_CORES, num_swdge_queues=4)
    xq = [nc.dram_tensor(f"xq{q}", [QL, D], f32, kind="ExternalInput")
          for q in range(4)]
    W1b = nc.dram_tensor("W1b", [128, 128], bf16, kind="ExternalInput")
    W2b = nc.dram_tensor("W2b", [128, 128], bf16, kind="ExternalInput")
    b1b = nc.dram_tensor("b1b", [128, 128], f32, kind="ExternalInput")
    b2b = nc.dram_tensor("b2b", [128, 128], f32, kind="ExternalInput")
    iotab = nc.dram_tensor("iotab", [128, 128], bf16, kind="ExternalInput")
    gx = nc.dram_tensor("gx", [128, sch["tot"] // 16], i16,
                        kind="ExternalInput")
    dS = nc.dram_tensor("dS", [128, sch["n_ops"]], f32, kind="ExternalInput")
    dd = nc.dram_tensor("dd", [128, sch["nch"]], f32, kind="ExternalInput")
    out = nc.dram_tensor("out", [S, D], f32, kind="ExternalOutput")

    agin = nc.dram_tensor("agin", [SP, D], bf16)
    agout = nc.dram_tensor("agout", [NPAD, D], bf16,
                           addr_space="Shared" if shared else "Local")

    wpwb = [min(NW - bb * WPT, WPT) for bb in range(NWB)]

    with tile.TileContext(nc) as tc:
        with (
            tc.tile_pool(name="cst", bufs=1) as cst,
            tc.tile_pool(name="ps", bufs=2, space="PSUM") as php,
            tc.tile_pool(name="st", bufs=4) as stp,
            tc.tile_pool(name="stc", bufs=9) as stcp,
            tc.tile_pool(name="oh", bufs=4) as ohp,
            tc.tile_pool(name="bank", bufs=1, space="PSUM") as bankp,
            tc.tile_pool(name="fl", bufs=4) as flp,
            tc.tile_pool(name="zb", bufs=1) as zbp,
        ):
            nc.gpsimd.load_library(library_config.mlp)

            W1_sb = cst.tile([128, 128], bf16, tag="W1")
            W2_sb = cst.tile([128, 128], bf16, tag="W2")
            b1_sb = cst.tile([128, 128], f32, tag="b1")
            b2_sb = cst.tile([128, 128], f32, tag="b2")
            iota_sb = cst.tile([128, 128], bf16, tag="iota")
            gx_sb = cst.tile([128, sch["tot"] // 16], i16, tag="gx")
            dS_sb = cst.tile([128, sch["n_ops"]], f32, tag="dS")
            dd_sb = cst.tile([128, sch["nch"]], f32, tag="dd")
            nc.sync.dma_start(W1_sb[:], W1b[:])
            nc.sync.dma_start(W2_sb[:], W2b[:])
            nc.sync.dma_start(b1_sb[:], b1b[:])
            nc.sync.dma_start(b2_sb[:], b2b[:])
            nc.sync.dma_start(iota_sb[:], iotab[:])
            nc.sync.dma_start(gx_sb[:], gx[:])
            nc.sync.dma_start(dS_sb[:], dS[:])
            nc.sync.dma_start(dd_sb[:], dd[:])
            banks = [bankp.tile([128, 512], f32, tag=f"bk{i}",
                                name=f"bank{i}") for i in range(6)]
            zpre = zbp.tile([128, SP], bf16, tag="zpre", name="zpre")
            agg2 = zbp.tile([128, SP], bf16, tag="agg2", name="agg2")

            qctr = [0]

            def aggregate(layer1):
                """Gather + one-hot-matmul aggregation over own dst nodes.
                layer1: gathers f32 x quarters, casts to bf16, accumulates
                [feat, dst] into banks, flushes to zpre + dense W1 -> agin.
                else: gathers bf16 agout, accumulates, flush to agg2 + dense
                W2 -> out."""
                for bb in range(NWB):
                    for qq in range(4):
                        Lseg = sch["L"][(bb, qq)]
                        if Lseg == 0:
                            continue
                        base = sch["seg_base"][(bb, qq)]
                        ncalls = (Lseg + CALL - 1) // CALL
                        assert ncalls <= 9, (Lseg, CALL)
                        stages = []
                        for k in range(ncalls):
                            cl = min(CALL, Lseg - CALL * k)
                            if layer1:
                                stg = stp.tile([128, CALL // 128, 128], f32,
                                               tag="stg")
                                nc.gpsimd.dma_gather(
                                    stg[:, :cl // 128, :], xq[qq].ap(),
                                    gx_sb[:, (base + CALL * k) // 16:
                                          (base + CALL * k + cl) // 16],
                                    cl, cl, 128, queue_num=qctr[0] % 4)
                                qctr[0] += 1
                                stb = stcp.tile([128, CALL // 128, 128],
                                                bf16, tag="stb")
                                nc.vector.tensor_copy(
                                    stb[:, :cl // 128, :],
                                    stg[:, :cl // 128, :])
                            else:
                                stb = stcp.tile([128, CALL // 128, 128],
                                                bf16, tag="stb")
                                nc.gpsimd.dma_gather(
                                    stb[:, :cl // 128, :],
                                    agout.ap()[QL * qq:QL * (qq + 1), :],
                                    gx_sb[:, (base + CALL * k) // 16:
                                          (base + CALL * k + cl) // 16],
                                    cl, cl, 128, queue_num=qctr[0] % 4)
                                qctr[0] += 1
                            stages.append(stb)
                        for op in sch["ops_by_seg"][(bb, qq)]:
                            _, _, jseg, jg, wv, oi, st_f, sp_f = op
                            k = jseg // (CALL // 128)
                            jc = jseg % (CALL // 128)
                            wl = wv - bb * WPT
                            bank = banks[(bb % 2) * 3 + wl // 4]
                            bsl = bank[:, 128 * (wl % 4):128 * (wl % 4 + 1)]
                            S_t = ohp.tile([128, 128], bf16, tag="S")
                            nc.vector.tensor_scalar(
                                S_t[:], iota_sb[:],
                                dS_sb[:, oi:oi + 1], dd_sb[:, jg:jg + 1],
                                op0=mybir.AluOpType.is_equal,
                                op1=mybir.AluOpType.mult)
                            nc.tensor.matmul(
                                bsl, lhsT=stages[k][:, jc, :], rhs=S_t[:],
                                start=st_f, stop=sp_f)
                    # flush block: dense transform per window
                    nwin = wpwb[bb]
                    acc = zpre if layer1 else agg2
                    for wl in range(nwin):
                        wv = bb * WPT + wl
                        bank = banks[(bb % 2) * 3 + wl // 4]
                        bsl = bank[:, 128 * (wl % 4):128 * (wl % 4 + 1)]
                        nc.vector.tensor_copy(
                            acc[:, 128 * wv:128 * (wv + 1)], bsl)
                    w_sb = W1_sb if layer1 else W2_sb
                    b_sb = b1_sb if layer1 else b2_sb
                    ob = flp.tile([128, WPT * 128], bf16 if layer1 else f32,
                                  tag="ob1" if layer1 else "ob2")
                    for wl in range(nwin):
                        wv = bb * WPT + wl
                        ps = php.tile([128, 128], f32, tag="php")
                        nc.tensor.matmul(
                            ps[:], lhsT=acc[:, 128 * wv:128 * (wv + 1)],
                            rhs=w_sb[:], start=True, stop=True)
                        osl = ob[:, 128 * wl:128 * (wl + 1)]
                        if layer1:
                            # relu(ps + b1) -> bf16
                            t1 = flp.tile([128, 128], f32, tag="t1")
                            nc.vector.tensor_tensor(
                                t1[:], ps[:], b_sb[:],
                                op=mybir.AluOpType.add)
                            nc.vector.tensor_scalar(
                                osl, t1[:], 0.0, None,
                                op0=mybir.AluOpType.max)
                        else:
                            nc.vector.tensor_tensor(
                                osl, ps[:], b_sb[:],
                                op=mybir.AluOpType.add)
                    w0 = bb * WPT
                    if layer1:
                        nc.sync.dma_start(
                            agin.ap()[128 * w0:128 * (w0 + nwin), :]
                            .rearrange("(t p) f -> p t f", p=128),
                            ob[:, :128 * nwin].rearrange(
                                "p (t f) -> p t f", f=128))
                    else:
                        rows = min(S, 128 * (w0 + nwin)) - 128 * w0
                        nfull = rows // 128
                        if nfull:
                            nc.sync.dma_start(
                                out.ap()[128 * w0:128 * (w0 + nfull), :]
                                .rearrange("(t p) f -> p t f", p=128),
                                ob[:, :128 * nfull].rearrange(
                                    "p (t f) -> p t f", f=128))
                        rem = rows - 128 * nfull
                        if rem:
                            nc.sync.dma_start(
                                out.ap()[128 * (w0 + nfull):
                                         128 * (w0 + nfull) + rem, :],
                                ob[0:rem, 128 * nfull:128 * (nfull + 1)])

            for r in range(repeat):
                if "B" in phases:
                    with nc.named_scope("phB"):
                        aggregate(layer1=True)
                if ag:
                    with nc.named_scope("phAG"):
                        nc.gpsimd.collective_compute(
                            "AllGather", mybir.AluOpType.bypass,
                            replica_groups=[list(range(N_CORES))],
                            ins=[agin[:].opt()], outs=[agout[:].opt()])
                if "D" in phases:
                    with nc.named_scope("phD"):
                        aggregate(layer1=False)

    nc.compile()
    return nc


def _make_in_maps(x, W1, b1, W2, b2, sch):
    xp = np.zeros((NPAD, D), np.float32)
    xv = np.asarray(x, np.float32)
    for c in range(N_CORES):
        xp[c * SP:c * SP + S] = xv[c * S:(c + 1) * S]
    iota = np.broadcast_to(np.arange(128, dtype=np.float32),
                           (128, 128)).astype(ml_dtypes.bfloat16)
    common = {
        **{f"xq{q}": np.ascontiguousarray(xp[QL * q:QL * (q + 1)])
           for q in range(4)},
        "W1b": np.asarray(W1, np.float32).astype(ml_dtypes.bfloat16),
        "W2b": np.asarray(W2, np.float32).astype(ml_dtypes.bfloat16),
        "b1b": np.broadcast_to(np.asarray(b1, np.float32), (128, 128)).copy(),
        "b2b": np.broadcast_to(np.asarray(b2, np.float32), (128, 128)).copy(),
        "iotab": np.ascontiguousarray(iota),
    }
    in_maps = []
    for c in range(N_CORES):
        in_maps.append({
            **common,
            "gx": _wrap_idx(sch["gidx"][c]),
            "dS": sch["dS"][c],
            "dd": sch["dd"][c],
        })
    return in_maps


def kernel(x, edge_index, W1, b1, W2, b2):
    sch = _prep(edge_index)
    nc = _build(sch, repeat=int(os.environ.get("KERNEL_REPEAT", "1")))
    in_maps = _make_in_maps(x, W1, b1, W2, b2, sch)
    res = run_bass_kernel_spmd(nc, in_maps, core_ids=list(range(N_CORES)))
    return np.concatenate([res.results[c]["out"] for c in range(N_CORES)], 0)


# revision 18
# speedup vs baseline: 2.5083x; 1.0451x over previous
"""GCN 2-layer encoder on 8 Trainium2 NeuronCores — sharded design.

Key identity: segment_sum(norm * (x@W)[src]) == segment_sum(norm * x[src]) @ W,
so each layer gathers RAW features per edge (full per-edge norm fused into the
one-hot scatter matrix), aggregates into [own_nodes, 128], and applies the
dense 128x128 transform AFTER aggregation (8x less dense work, no gather-table
build for layer 1 at all: layer 1 gathers x directly as f32 512B rows = full
DMA descriptor rate).

Each core aggregates only its OWN 12500 dst nodes for both layers (~215k edges
per core instead of 1.7M). Between the layers one AllGather shares the z
activations (25.6MB bf16, ~250us measured on this runtime). Both layers reuse
the SAME edge schedule / gather indices / norm tables (identical edge set).

Node relabeling: new_id(g) = (g//12500)*12544 + g%12500 so each core's slice
is window-aligned (98 windows of 128). Gather tables are addressed per
quarter (25088 rows) to satisfy dma_gather's int16 index requirement.

Timing note: collectives cannot sit inside a hardware For_i loop, so the
repeat loop is Python-unrolled.
"""
import os
import sys

sys.path.insert(0, "/opt/trn_rl_repo")
import numpy as np
import ml_dtypes

import concourse.tile as tile
from concourse import bacc, mybir, library_config
from concourse.bass_utils import run_bass_kernel_spmd

N_NODES = 100000
N_CORES = 8
S = N_NODES // N_CORES          # 12500 own nodes per core
SP = 12544                      # padded own slice (98 windows x 128)
D = 128
NPAD = SP * N_CORES             # 100352 relabeled table rows
QL = 25088                      # quarter length (int16-safe)
NW = SP // 128                  # 98 own dst windows
WPT = 12                        # windows per block (3 PSUM banks x 4)
NWB = (NW + WPT - 1) // WPT     # 9 blocks
CALL = int(os.environ.get("KERNEL_CALL", "1024"))
f32 = mybir.dt.float32
bf16 = mybir.dt.bfloat16
i16 = mybir.dt.int16


def _pad128(n):
    return max(128, ((n + 127) // 128) * 128)


def _wrap_idx(gidx_flat):
    """[slots] int16 -> [128, slots/16] wrapped+replicated for dma_gather."""
    a = gidx_flat.reshape(-1, 16).T
    return np.tile(a, (8, 1)).copy()


def _sched_core(src_n, dst_l, w_norm):
    """Schedule one core's edges: segment by (block of dst window, quarter of
    src), pad each segment to x128, return slot arrays + op list.

    src_n: relabeled global src ids; dst_l: local dst (0..12499);
    w_norm: per-edge norm (f64).
    """
    w = dst_l // 128
    b = w // WPT
    q = src_n // QL
    order = np.lexsort((src_n, w, q, b))
    so, do, qo, bo = src_n[order], dst_l[order], q[order], b[order]
    no = w_norm[order]
    segkey = bo * 4 + qo
    n = len(so)
    bounds = np.flatnonzero(np.diff(segkey)) + 1
    starts = np.concatenate([[0], bounds])
    ends = np.concatenate([bounds, [n]])
    seg_edges = {int(segkey[s0]): (s0, e0) for s0, e0 in zip(starts, ends)}

    seg_list = [(bb, qq) for bb in range(NWB) for qq in range(4)]
    L = {}
    for (bb, qq) in seg_list:
        k = bb * 4 + qq
        ln = seg_edges[k][1] - seg_edges[k][0] if k in seg_edges else 0
        L[(bb, qq)] = _pad128(ln) if ln else 0
    tot = sum(L.values())
    gidx = np.zeros(tot, np.int16)
    dstv = np.full(tot, -1.0e6, np.float64)
    ddv = np.zeros(tot, np.float64)
    seg_base = {}
    off = 0
    for (bb, qq) in seg_list:
        seg_base[(bb, qq)] = off
        k = bb * 4 + qq
        if k in seg_edges:
            s0, e0 = seg_edges[k]
            m = e0 - s0
            gidx[off:off + m] = (so[s0:e0] - QL * qo[s0]).astype(np.int16)
            dstv[off:off + m] = do[s0:e0]
            ddv[off:off + m] = no[s0:e0]
        off += L[(bb, qq)]
    return {"L": L, "seg_base": seg_base, "tot": tot,
            "gidx": gidx, "dstv": dstv, "ddv": ddv}


def _prep(edge_index):
    """Returns unified per-core schedule dict."""
    src = np.asarray(edge_index[0], dtype=np.int64)
    dst = np.asarray(edge_index[1], dtype=np.int64)
    deg = (np.bincount(dst, minlength=N_NODES) + 1).astype(np.float64)
    dinv = 1.0 / np.sqrt(deg)

    loop = np.arange(N_NODES, dtype=np.int64)
    src_all = np.concatenate([src, loop])
    dst_all = np.concatenate([dst, loop])
    norm_all = dinv[src_all] * dinv[dst_all]
    src_new = (src_all // S) * SP + (src_all % S)

    core = dst_all // S
    percore = []
    for c in range(N_CORES):
        m = core == c
        percore.append(_sched_core(src_new[m], dst_all[m] - c * S,
                                   norm_all[m]))

    # unify segment lengths across cores
    seg_list = [(bb, qq) for bb in range(NWB) for qq in range(4)]
    Lc = {sk: max(p["L"][sk] for p in percore) for sk in seg_list}
    tot = sum(Lc.values())
    nch = tot // 128
    seg_base = {}
    off = 0
    for sk in seg_list:
        seg_base[sk] = off
        off += Lc[sk]
    gidx = np.zeros((N_CORES, tot), np.int16)
    dstv = np.full((N_CORES, tot), -1.0e6, np.float64)
    ddv = np.zeros((N_CORES, tot), np.float64)
    for c in range(N_CORES):
        p = percore[c]
        for sk in seg_list:
            ln = p["L"][sk]
            if ln == 0:
                continue
            sb_c, sb_u = p["seg_base"][sk], seg_base[sk]
            gidx[c, sb_u:sb_u + ln] = p["gidx"][sb_c:sb_c + ln]
            dstv[c, sb_u:sb_u + ln] = p["dstv"][sb_c:sb_c + ln]
            ddv[c, sb_u:sb_u + ln] = p["ddv"][sb_c:sb_c + ln]

    # ops: union over cores of (chunk, window) pairs with any real slot
    slot_ch = np.arange(tot) // 128
    real = dstv >= 0
    wl_slot = np.where(real, (dstv // 128).astype(np.int64), 0)
    opkey = slot_ch[None, :] * NW + wl_slot
    op_keys = np.unique(opkey[real])
    n_ops = len(op_keys)
    op_of = np.searchsorted(op_keys, opkey)
    # per-op dS (dst - 128*w for slots in that op's window, else -1000)
    dS = np.full((N_CORES, n_ops, 128), -1000.0, np.float32)
    for c in range(N_CORES):
        sl = np.flatnonzero(real[c])
        dS[c, op_of[c, sl], sl % 128] = \
            dstv[c, sl] - 128.0 * wl_slot[c, sl]
    dS = np.ascontiguousarray(dS.transpose(0, 2, 1))      # [C,128,n_ops]
    ddT = np.zeros((N_CORES, nch, 128), np.float32)
    for c in range(N_CORES):
        sl = np.flatnonzero(real[c])
        ddT[c, slot_ch[sl], sl % 128] = ddv[c, sl]
    ddT = np.ascontiguousarray(ddT.transpose(0, 2, 1))    # [C,128,nch]

    # op metadata in emission order (seg-major), interleaved across windows
    op_ch = op_keys // NW
    op_w = op_keys % NW
    ops_by_seg = {sk: [] for sk in seg_list}
    for oi in range(n_ops):
        j = int(op_ch[oi])
        wv = int(op_w[oi])
        base = j * 128
        bb = wv // WPT
        qq = None
        for qx in range(4):
            sb = seg_base[(bb, qx)]
            if sb <= base < sb + Lc[(bb, qx)]:
                qq = qx
                break
        assert qq is not None, (j, wv, bb)
        jseg = (base - seg_base[(bb, qq)]) // 128
        ops_by_seg[(bb, qq)].append([bb, qq, jseg, j, wv, oi, False, False])
    # interleave ops across windows within each segment (PSUM bank spread)
    for sk in seg_list:
        lst = ops_by_seg[sk]
        cnt = {}
        keyed = []
        for op in lst:
            r = cnt.get(op[4], 0)
            cnt[op[4]] = r + 1
            wl = op[4] - op[0] * WPT
            keyed.append((r, wl % 4, wl // 4, op))
        ops_by_seg[sk] = [t[3] for t in
                          sorted(keyed, key=lambda t: (t[0], t[1], t[2]))]
    # start/stop flags per (block, bank) in final emission order
    flat = []
    for sk in seg_list:
        flat.extend(ops_by_seg[sk])
    first_bk, last_bk = {}, {}
    for i, op in enumerate(flat):
        bk = (op[0], (op[4] - op[0] * WPT) // 4)
        if bk not in first_bk:
            first_bk[bk] = i
        last_bk[bk] = i
    for i in first_bk.values():
        flat[i][6] = True
    for i in last_bk.values():
        flat[i][7] = True
    # emission-ordered dS (bf16) for batched equality builds
    order_oi = np.array([op[5] for op in flat], np.int64)
    dSem = np.ascontiguousarray(dS[:, :, order_oi]).astype(ml_dtypes.bfloat16)
    return {
        "L": Lc, "seg_base": seg_base, "seg_list": seg_list,
        "tot": tot, "nch": nch, "n_ops": n_ops,
        "gidx": gidx, "dSem": dSem, "dd": ddT, "ops_by_seg": ops_by_seg,
    }


def _build(sch, repeat=1, phases="BD", ag=True, shared=True):
    nc = bacc.Bacc("TRN2", target_bir_lowering=False, debug=False,
                   num_devices=N_CORES, num_swdge_queues=4)
    xq = [nc.dram_tensor(f"xq{q}", [QL, D], f32, kind="ExternalInput")
          for q in range(4)]
    W1b = nc.dram_tensor("W1b", [128, 128], bf16, kind="ExternalInput")
    W2b = nc.dram_tensor("W2b", [128, 128], bf16, kind="ExternalInput")
    b1b = nc.dram_tensor("b1b", [128, 128], f32, kind="ExternalInput")
    b2b = nc.dram_tensor("b2b", [128, 128], f32, kind="ExternalInput")
    iotab = nc.dram_tensor("iotab", [128, 128], bf16, kind="ExternalInput")
    gx = nc.dram_tensor("gx", [128, sch["tot"] // 16], i16,
                        kind="ExternalInput")
    dSe = nc.dram_tensor("dSe", [128, sch["n_ops"]], bf16,
                         kind="ExternalInput")
    dd = nc.dram_tensor("dd", [128, sch["nch"]], f32, kind="ExternalInput")
    out = nc.dram_tensor("out", [S, D], f32, kind="ExternalOutput")

    agin = nc.dram_tensor("agin", [SP, D], bf16)
    agout = nc.dram_tensor("agout", [NPAD, D], bf16,
                           addr_space="Shared" if shared else "Local")

    wpwb = [min(NW - bb * WPT, WPT) for bb in range(NWB)]

    with tile.TileContext(nc) as tc:
        with (
            tc.tile_pool(name="cst", bufs=1) as cst,
            tc.tile_pool(name="ps", bufs=2, space="PSUM") as php,
            tc.tile_pool(name="st", bufs=3) as stp,
            tc.tile_pool(name="stc", bufs=9) as stcp,
            tc.tile_pool(name="oh", bufs=3) as ohp,
            tc.tile_pool(name="bank", bufs=1, space="PSUM") as bankp,
            tc.tile_pool(name="fl", bufs=2) as flp,
            tc.tile_pool(name="zb", bufs=1) as zbp,
        ):
            nc.gpsimd.load_library(library_config.mlp)

            W1_sb = cst.tile([128, 128], bf16, tag="W1")
            W2_sb = cst.tile([128, 128], bf16, tag="W2")
            b1_sb = cst.tile([128, 128], f32, tag="b1")
            b2_sb = cst.tile([128, 128], f32, tag="b2")
            iota_sb = cst.tile([128, 128], bf16, tag="iota")
            gx_sb = cst.tile([128, sch["tot"] // 16], i16, tag="gx")
            dS_sb = cst.tile([128, sch["n_ops"]], bf16, tag="dS")
            dd_sb = cst.tile([128, sch["nch"]], f32, tag="dd")
            nc.sync.dma_start(W1_sb[:], W1b[:])
            nc.sync.dma_start(W2_sb[:], W2b[:])
            nc.sync.dma_start(b1_sb[:], b1b[:])
            nc.sync.dma_start(b2_sb[:], b2b[:])
            nc.sync.dma_start(iota_sb[:], iotab[:])
            nc.sync.dma_start(gx_sb[:], gx[:])
            nc.sync.dma_start(dS_sb[:], dSe[:])
            nc.sync.dma_start(dd_sb[:], dd[:])
            banks = [bankp.tile([128, 512], f32, tag=f"bk{i}",
                                name=f"bank{i}") for i in range(6)]
            zpre = zbp.tile([128, SP], bf16, tag="zpre", name="zpre")
            agg2 = zbp.tile([128, SP], bf16, tag="agg2", name="agg2")

            qctr = [0]

            ectr = [0]   # emission-ordered op counter (shared by layers)

            def aggregate(layer1):
                """Gather + one-hot-matmul aggregation over own dst nodes.
                Per-edge norm is folded into the stage tiles (mult-cast with
                dd broadcast); S matrices are pure-equality one-hots built in
                batches of G ops with a single broadcast tensor_tensor."""
                G = 16
                ectr[0] = 0
                for bb in range(NWB):
                    for qq in range(4):
                        Lseg = sch["L"][(bb, qq)]
                        if Lseg == 0:
                            continue
                        base = sch["seg_base"][(bb, qq)]
                        ncalls = (Lseg + CALL - 1) // CALL
                        assert ncalls <= 9, (Lseg, CALL)
                        stages = []
                        for k in range(ncalls):
                            cl = min(CALL, Lseg - CALL * k)
                            nc_ = cl // 128
                            ch0 = (base + CALL * k) // 128
                            dd_b = dd_sb[:, ch0:ch0 + nc_] \
                                .unsqueeze(2).broadcast_to([128, nc_, 128])
                            if layer1:
                                stg = stp.tile([128, CALL // 128, 128], f32,
                                               tag="stg")
                                nc.gpsimd.dma_gather(
                                    stg[:, :nc_, :], xq[qq].ap(),
                                    gx_sb[:, (base + CALL * k) // 16:
                                          (base + CALL * k + cl) // 16],
                                    cl, cl, 128, queue_num=qctr[0] % 4)
                            else:
                                stg = stp.tile([128, CALL // 128, 128], bf16,
                                               tag="str")
                                nc.gpsimd.dma_gather(
                                    stg[:, :nc_, :],
                                    agout.ap()[QL * qq:QL * (qq + 1), :],
                                    gx_sb[:, (base + CALL * k) // 16:
                                          (base + CALL * k + cl) // 16],
                                    cl, cl, 128, queue_num=qctr[0] % 4)
                            qctr[0] += 1
                            stb = stcp.tile([128, CALL // 128, 128],
                                            bf16, tag="stb")
                            nc.vector.tensor_tensor(
                                stb[:, :nc_, :], stg[:, :nc_, :], dd_b,
                                op=mybir.AluOpType.mult)
                            stages.append(stb)
                        ops = sch["ops_by_seg"][(bb, qq)]
                        sbatch = None
                        for ei, op in enumerate(ops):
                            _, _, jseg, jg, wv, oi, st_f, sp_f = op
                            li = ei % G
                            if li == 0:
                                g = min(G, len(ops) - ei)
                                e0 = ectr[0] + ei
                                sbatch = ohp.tile([128, G, 128], bf16,
                                                  tag="S")
                                io_b = iota_sb[:].unsqueeze(1) \
                                    .broadcast_to([128, g, 128])
                                dS_b = dS_sb[:, e0:e0 + g].unsqueeze(2) \
                                    .broadcast_to([128, g, 128])
                                nc.vector.tensor_tensor(
                                    sbatch[:, :g, :], io_b, dS_b,
                                    op=mybir.AluOpType.is_equal)
                            k = jseg // (CALL // 128)
                            jc = jseg % (CALL // 128)
                            wl = wv - bb * WPT
                            bank = banks[(bb % 2) * 3 + wl // 4]
                            bsl = bank[:, 128 * (wl % 4):128 * (wl % 4 + 1)]
                            nc.tensor.matmul(
                                bsl, lhsT=stages[k][:, jc, :],
                                rhs=sbatch[:, li, :],
                                start=st_f, stop=sp_f)
                        ectr[0] += len(ops)
                    # flush block: dense transform per window
                    nwin = wpwb[bb]
                    acc = zpre if layer1 else agg2
                    for wl in range(nwin):
                        wv = bb * WPT + wl
                        bank = banks[(bb % 2) * 3 + wl // 4]
                        bsl = bank[:, 128 * (wl % 4):128 * (wl % 4 + 1)]
                        nc.scalar.activation(
                            acc[:, 128 * wv:128 * (wv + 1)], bsl,
                            mybir.ActivationFunctionType.Copy)
                    w_sb = W1_sb if layer1 else W2_sb
                    b_sb = b1_sb if layer1 else b2_sb
                    ob = flp.tile([128, WPT * 128], bf16 if layer1 else f32,
                                  tag="ob1" if layer1 else "ob2")
                    for wl in range(nwin):
                        wv = bb * WPT + wl
                        ps = php.tile([128, 128], f32, tag="php")
                        nc.tensor.matmul(
                            ps[:], lhsT=acc[:, 128 * wv:128 * (wv + 1)],
                            rhs=w_sb[:], start=True, stop=True)
                        osl = ob[:, 128 * wl:128 * (wl + 1)]
                        if layer1:
                            # relu(ps + b1) -> bf16
                            t1 = flp.tile([128, 128], f32, tag="t1")
                            nc.vector.tensor_tensor(
                                t1[:], ps[:], b_sb[:],
                                op=mybir.AluOpType.add)
                            nc.vector.tensor_scalar(
                                osl, t1[:], 0.0, None,
                                op0=mybir.AluOpType.max)
                        else:
                            nc.vector.tensor_tensor(
                                osl, ps[:], b_sb[:],
                                op=mybir.AluOpType.add)
                    w0 = bb * WPT
                    if layer1:
                        nc.sync.dma_start(
                            agin.ap()[128 * w0:128 * (w0 + nwin), :]
                            .rearrange("(t p) f -> p t f", p=128),
                            ob[:, :128 * nwin].rearrange(
                                "p (t f) -> p t f", f=128))
                    else:
                        rows = min(S, 128 * (w0 + nwin)) - 128 * w0
                        nfull = rows // 128
                        if nfull:
                            nc.sync.dma_start(
                                out.ap()[128 * w0:128 * (w0 + nfull), :]
                                .rearrange("(t p) f -> p t f", p=128),
                                ob[:, :128 * nfull].rearrange(
                                    "p (t f) -> p t f", f=128))
                        rem = rows - 128 * nfull
                        if rem:
                            nc.sync.dma_start(
                                out.ap()[128 * (w0 + nfull):
                                         128 * (w0 + nfull) + rem, :],
                                ob[0:rem, 128 * nfull:128 * (nfull + 1)])

            for r in range(repeat):
                if "B" in phases:
                    with nc.named_scope("phB"):
                        aggregate(layer1=True)
                if ag:
                    with nc.named_scope("phAG"):
                        nc.gpsimd.collective_compute(
                            "AllGather", mybir.AluOpType.bypass,
                            replica_groups=[list(range(N_CORES))],
                            ins=[agin[:].opt()], outs=[agout[:].opt()])
                if "D" in phases:
                    with nc.named_scope("phD"):
                        aggregate(layer1=False)

    nc.compile()
    return nc


def _make_in_maps(x, W1, b1, W2, b2, sch):
    xp = np.zeros((NPAD, D), np.float32)
    xv = np.asarray(x, np.float32)
    for c in range(N_CORES):
        xp[c * SP:c * SP + S] = xv[c * S:(c + 1) * S]
    iota = np.broadcast_to(np.arange(128, dtype=np.float32),
                           (128, 128)).astype(ml_dtypes.bfloat16)
    common = {
        **{f"xq{q}": np.ascontiguousarray(xp[QL * q:QL * (q + 1)])
           for q in range(4)},
        "W1b": np.asarray(W1, np.float32).astype(ml_dtypes.bfloat16),
        "W2b": np.asarray(W2, np.float32).astype(ml_dtypes.bfloat16),
        "b1b": np.broadcast_to(np.asarray(b1, np.float32), (128, 128)).copy(),
        "b2b": np.broadcast_to(np.asarray(b2, np.float32), (128, 128)).copy(),
        "iotab": np.ascontiguousarray(iota),
    }
    in_maps = []
    for c in range(N_CORES):
        in_maps.append({
            **common,
            "gx": _wrap_idx(sch["gidx"][c]),
            "dSe": sch["dSem"][c],
            "dd": sch["dd"][c],
        })
    return in_maps


def kernel(x, edge_index, W1, b1, W2, b2):
    sch = _prep(edge_index)
    nc = _build(sch, repeat=int(os.environ.get("KERNEL_REPEAT", "1")))
    in_maps = _make_in_maps(x, W1, b1, W2, b2, sch)
    res = run_bass_kernel_spmd(nc, in_maps, core_ids=list(range(N_CORES)))
    return np.concatenate([res.results[c]["out"] for c in range(N_CORES)], 0)


# revision 24
# speedup vs baseline: 3.0733x; 1.2253x over previous
"""GCN 2-layer encoder on 8 Trainium2 NeuronCores — sharded design v4.

Key identity: segment_sum(norm * (x@W)[src]) == segment_sum(norm * x[src]) @ W,
so each layer gathers RAW features per edge (per-edge norm folded into the
gathered stage tiles via a broadcast multiply), aggregates into
[own_nodes, 128] via one-hot matmuls, and applies the dense 128x128 transform
AFTER aggregation. Layer 1 gathers x directly as f32 512B rows (full DMA
descriptor rate, no table build); layer 2 gathers the allgathered bf16 z.

Each core aggregates only its OWN 12500 dst nodes (~215k edges) for both
layers. Between the layers the z activations are shared with FOUR chunked
AllGathers (6.5MB each) so the collectives overlap with the tail of layer-1
compute and the head of layer-2 gathering.

Schedule: per-core edges bucketed by (dst window, src chunk) with per-window
x128 padding unified across cores, so every 128-slot chunk belongs to exactly
one dst window (one matmul per chunk, no per-op tables). Within a segment
chunks are round-robin interleaved across the block's 12 windows so
consecutive matmuls hit different PSUM banks. One-hot S matrices are built in
batches of G chunks with a single broadcast is_equal tensor_tensor.

Node relabeling: own slice padded to 12544 rows (98 windows), split into 4
AG chunks of [25,25,24,24] windows; table position of node (c,l) =
base_k + c*rows_k + (l - rstart_k). Gather indices are int16 offsets within
one AG chunk's table (max 25600 < 32767).

Collectives cannot sit inside a hardware For_i loop, so the repeat loop is
Python-unrolled.
"""
import os
import sys

sys.path.insert(0, "/opt/trn_rl_repo")
import numpy as np
import ml_dtypes

import concourse.tile as tile
from concourse import bacc, mybir, library_config
from concourse.bass_utils import run_bass_kernel_spmd

N_NODES = 100000
N_CORES = 8
S = N_NODES // N_CORES          # 12500 own nodes per core
SP = 12544                      # padded own slice (98 windows x 128)
D = 128
NW = SP // 128                  # 98 own dst windows
WPT = 12                        # windows per block (3 PSUM banks x 4)
NWB = (NW + WPT - 1) // WPT     # 9 blocks
NK = 4                          # AG chunks
WK = [25, 25, 24, 24]           # windows per AG chunk
WSTART = [0, 25, 50, 74]
ROWS_K = [w * 128 for w in WK]              # per-core rows per chunk
RSTART = [0, 3200, 6400, 9472]
BASE_K = [0, 25600, 51200, 75776]           # global table base per chunk
CALL = int(os.environ.get("KERNEL_CALL", "1024"))
G = 16                          # one-hot build batch (chunks per DVE op)
f32 = mybir.dt.float32
bf16 = mybir.dt.bfloat16
i16 = mybir.dt.int16


def _pad128(n):
    return ((n + 127) // 128) * 128


def _wrap_idx(gidx_flat):
    """[slots] int16 -> [128, slots/16] wrapped+replicated for dma_gather."""
    a = gidx_flat.reshape(-1, 16).T
    return np.tile(a, (8, 1)).copy()


def _chunk_of_local(l):
    """AG chunk index of own-local node l."""
    return np.searchsorted(RSTART, l, side="right") - 1


def _prep(edge_index):
    src = np.asarray(edge_index[0], dtype=np.int64)
    dst = np.asarray(edge_index[1], dtype=np.int64)
    deg = (np.bincount(dst, minlength=N_NODES) + 1).astype(np.float64)
    dinv = 1.0 / np.sqrt(deg)

    loop = np.arange(N_NODES, dtype=np.int64)
    src_all = np.concatenate([src, loop])
    dst_all = np.concatenate([dst, loop])
    norm_all = dinv[src_all] * dinv[dst_all]
    sc = src_all // S                       # src owner core
    sl = src_all % S                        # src local id
    sk = _chunk_of_local(sl)                # src AG chunk
    spos = np.take(BASE_K, sk) + sc * np.take(ROWS_K, sk) \
        + (sl - np.take(RSTART, sk))        # global table position
    sidx = spos - np.take(BASE_K, sk)       # int16 offset within chunk table

    core = dst_all // S
    # per (core, window, k): sorted edge lists
    per = []
    cnt = np.zeros((N_CORES, NW, NK), np.int64)
    for c in range(N_CORES):
        m = core == c
        dl = dst_all[m] - c * S
        w = dl // 128
        k = sk[m]
        order = np.lexsort((sidx[m], k, w))
        per.append({
            "w": w[order], "k": k[order], "dl": dl[order],
            "gi": sidx[m][order], "no": norm_all[m][order],
        })
        np.add.at(cnt[c], (w, k), 1)
    La = np.zeros((NW, NK), np.int64)
    for w in range(NW):
        for k in range(NK):
            La[w, k] = _pad128(int(cnt[:, w, k].max()))
    # chunk emission order per segment (bb, k): round-robin across windows
    seg_list = [(bb, k) for bb in range(NWB) for k in range(NK)]
    seg_base, seg_len = {}, {}
    chunk_meta = []                      # (bb, k, w, r) in emission order
    off = 0
    for (bb, k) in seg_list:
        seg_base[(bb, k)] = off
        wins = list(range(bb * WPT, min(NW, (bb + 1) * WPT)))
        nmax = max(La[w, k] // 128 for w in wins)
        cnt_emitted = 0
        for r in range(nmax):
            for w in wins:
                if r < La[w, k] // 128:
                    chunk_meta.append((bb, k, w, r))
                    cnt_emitted += 1
        ln = cnt_emitted * 128
        seg_len[(bb, k)] = ln
        off += ln
    tot = off
    nch = tot // 128
    # slot-fill per core
    gidx = np.zeros((N_CORES, tot), np.int16)
    dstm = np.full((N_CORES, nch, 128), -1000.0, np.float32)
    ddv = np.zeros((N_CORES, nch, 128), np.float32)
    # per-core edge run boundaries for each (w,k)
    for c in range(N_CORES):
        p = per[c]
        key = p["w"] * NK + p["k"]
        bounds = np.flatnonzero(np.diff(key)) + 1
        starts = np.concatenate([[0], bounds])
        ends = np.concatenate([bounds, [len(key)]])
        run = {int(key[s0]): (s0, e0) for s0, e0 in zip(starts, ends)}
        for ch, (bb, k, w, r) in enumerate(chunk_meta):
            kk = w * NK + k
            if kk not in run:
                continue
            s0, e0 = run[kk]
            a = s0 + r * 128
            b = min(s0 + (r + 1) * 128, e0)
            if a >= b:
                continue
            n = b - a
            gidx[c, ch * 128:ch * 128 + n] = p["gi"][a:b]
            dstm[c, ch, :n] = p["dl"][a:b] - 128.0 * w
            ddv[c, ch, :n] = p["no"][a:b]
    dSem = np.ascontiguousarray(
        dstm.transpose(0, 2, 1)).astype(ml_dtypes.bfloat16)   # [C,128,nch]
    ddT = np.ascontiguousarray(ddv.transpose(0, 2, 1))        # [C,128,nch]
    # ops = chunks; start/stop flags per (block, bank)
    ops_by_seg = {skey: [] for skey in seg_list}
    flags = {}
    for ch, (bb, k, w, r) in enumerate(chunk_meta):
        ops_by_seg[(bb, k)].append([ch, w, False, False])
    first_bk, last_bk = {}, {}
    i = 0
    for skey in seg_list:
        for op in ops_by_seg[skey]:
            ch, w = op[0], op[1]
            bb = skey[0]
            bk = (bb, (w - bb * WPT) // 4)
            if bk not in first_bk:
                first_bk[bk] = op
            last_bk[bk] = op
            i += 1
    for op in first_bk.values():
        op[2] = True
    for op in last_bk.values():
        op[3] = True
    return {
        "seg_list": seg_list, "seg_base": seg_base, "seg_len": seg_len,
        "tot": tot, "nch": nch,
        "gidx": gidx, "dSem": dSem, "dd": ddT, "ops_by_seg": ops_by_seg,
    }


def _build(sch, repeat=1, phases="BD", ag=True):
    nc = bacc.Bacc("TRN2", target_bir_lowering=False, debug=False,
                   num_devices=N_CORES, num_swdge_queues=4)
    xq = [nc.dram_tensor(f"xq{k}", [8 * ROWS_K[k], D], f32,
                         kind="ExternalInput") for k in range(NK)]
    W1b = nc.dram_tensor("W1b", [128, 128], bf16, kind="ExternalInput")
    W2b = nc.dram_tensor("W2b", [128, 128], bf16, kind="ExternalInput")
    b1b = nc.dram_tensor("b1b", [128, 128], f32, kind="ExternalInput")
    b2b = nc.dram_tensor("b2b", [128, 128], f32, kind="ExternalInput")
    iotab = nc.dram_tensor("iotab", [128, 128], bf16, kind="ExternalInput")
    gx = nc.dram_tensor("gx", [128, sch["tot"] // 16], i16,
                        kind="ExternalInput")
    dSe = nc.dram_tensor("dSe", [128, sch["nch"]], bf16,
                         kind="ExternalInput")
    dd = nc.dram_tensor("dd", [128, sch["nch"]], f32, kind="ExternalInput")
    out = nc.dram_tensor("out", [S, D], f32, kind="ExternalOutput")

    agin = [nc.dram_tensor(f"agin{k}", [ROWS_K[k], D], bf16)
            for k in range(NK)]
    agout = [nc.dram_tensor(f"agout{k}", [8 * ROWS_K[k], D], bf16,
                            addr_space="Shared") for k in range(NK)]

    wpwb = [min(NW - bb * WPT, WPT) for bb in range(NWB)]

    with tile.TileContext(nc) as tc:
        with (
            tc.tile_pool(name="cst", bufs=1) as cst,
            tc.tile_pool(name="ps", bufs=2, space="PSUM") as php,
            tc.tile_pool(name="st", bufs=4) as stp,
            tc.tile_pool(name="stc", bufs=12) as stcp,
            tc.tile_pool(name="oh", bufs=3) as ohp,
            tc.tile_pool(name="bank", bufs=1, space="PSUM") as bankp,
            tc.tile_pool(name="fl", bufs=2) as flp,
            tc.tile_pool(name="zb", bufs=1) as zbp,
        ):
            nc.gpsimd.load_library(library_config.mlp)

            W1_sb = cst.tile([128, 128], bf16, tag="W1")
            W2_sb = cst.tile([128, 128], bf16, tag="W2")
            b1_sb = cst.tile([128, 128], f32, tag="b1")
            b2_sb = cst.tile([128, 128], f32, tag="b2")
            iota_sb = cst.tile([128, 128], bf16, tag="iota")
            gx_sb = cst.tile([128, sch["tot"] // 16], i16, tag="gx")
            dS_sb = cst.tile([128, sch["nch"]], bf16, tag="dS")
            dd_sb = cst.tile([128, sch["nch"]], f32, tag="dd")
            nc.sync.dma_start(W1_sb[:], W1b[:])
            nc.sync.dma_start(W2_sb[:], W2b[:])
            nc.sync.dma_start(b1_sb[:], b1b[:])
            nc.sync.dma_start(b2_sb[:], b2b[:])
            nc.sync.dma_start(iota_sb[:], iotab[:])
            nc.sync.dma_start(gx_sb[:], gx[:])
            nc.sync.dma_start(dS_sb[:], dSe[:])
            nc.sync.dma_start(dd_sb[:], dd[:])
            banks = [bankp.tile([128, 512], f32, tag=f"bk{i}",
                                name=f"bank{i}") for i in range(6)]

            qctr = [0]

            def aggregate(layer1):
                acc = zbp.tile([128, SP], bf16, tag="acc")
                for bb in range(NWB):
                    for kq in range(NK):
                        Lseg = sch["seg_len"][(bb, kq)]
                        if Lseg == 0:
                            continue
                        base = sch["seg_base"][(bb, kq)]
                        ncalls = (Lseg + CALL - 1) // CALL
                        assert ncalls <= 13, (Lseg, CALL)
                        stages = []
                        for k in range(ncalls):
                            cl = min(CALL, Lseg - CALL * k)
                            nc_ = cl // 128
                            ch0 = (base + CALL * k) // 128
                            dd_b = dd_sb[:, ch0:ch0 + nc_] \
                                .unsqueeze(2).broadcast_to([128, nc_, 128])
                            if layer1:
                                stg = stp.tile([128, CALL // 128, 128], f32,
                                               tag="stg")
                                src_ap = xq[kq].ap()
                            else:
                                stg = stp.tile([128, CALL // 128, 128], bf16,
                                               tag="str")
                                src_ap = agout[kq].ap()
                            nc.gpsimd.dma_gather(
                                stg[:, :nc_, :], src_ap,
                                gx_sb[:, (base + CALL * k) // 16:
                                      (base + CALL * k + cl) // 16],
                                cl, cl, 128, queue_num=qctr[0] % 4)
                            qctr[0] += 1
                            stb = stcp.tile([128, CALL // 128, 128],
                                            bf16, tag="stb")
                            nc.vector.tensor_tensor(
                                stb[:, :nc_, :], stg[:, :nc_, :], dd_b,
                                op=mybir.AluOpType.mult)
                            stages.append(stb)
                        ops = sch["ops_by_seg"][(bb, kq)]
                        sbatch = None
                        for ei, op in enumerate(ops):
                            ch, wv, st_f, sp_f = op
                            li = ei % G
                            if li == 0:
                                g = min(G, len(ops) - ei)
                                ch0 = base // 128 + ei
                                sbatch = ohp.tile([128, G, 128], bf16,
                                                  tag="S")
                                io_b = iota_sb[:].unsqueeze(1) \
                                    .broadcast_to([128, g, 128])
                                dS_b = dS_sb[:, ch0:ch0 + g].unsqueeze(2) \
                                    .broadcast_to([128, g, 128])
                                nc.vector.tensor_tensor(
                                    sbatch[:, :g, :], io_b, dS_b,
                                    op=mybir.AluOpType.is_equal)
                            jseg = ch - base // 128
                            k = jseg // (CALL // 128)
                            jc = jseg % (CALL // 128)
                            wl = wv - bb * WPT
                            bank = banks[(bb % 2) * 3 + wl // 4]
                            bsl = bank[:, 128 * (wl % 4):128 * (wl % 4 + 1)]
                            nc.tensor.matmul(
                                bsl, lhsT=stages[k][:, jc, :],
                                rhs=sbatch[:, li, :],
                                start=st_f, stop=sp_f)
                    # flush block: dense transform per window
                    nwin = wpwb[bb]
                    for wl in range(nwin):
                        wv = bb * WPT + wl
                        bank = banks[(bb % 2) * 3 + wl // 4]
                        bsl = bank[:, 128 * (wl % 4):128 * (wl % 4 + 1)]
                        nc.scalar.activation(
                            acc[:, 128 * wv:128 * (wv + 1)], bsl,
                            mybir.ActivationFunctionType.Copy)
                    w_sb = W1_sb if layer1 else W2_sb
                    b_sb = b1_sb if layer1 else b2_sb
                    ob = flp.tile([128, WPT * 128], bf16 if layer1 else f32,
                                  tag="ob1" if layer1 else "ob2")
                    for wl in range(nwin):
                        wv = bb * WPT + wl
                        ps = php.tile([128, 128], f32, tag="php")
                        nc.tensor.matmul(
                            ps[:], lhsT=acc[:, 128 * wv:128 * (wv + 1)],
                            rhs=w_sb[:], start=True, stop=True)
                        osl = ob[:, 128 * wl:128 * (wl + 1)]
                        if layer1:
                            t1 = flp.tile([128, 128], f32, tag="t1")
                            nc.vector.tensor_tensor(
                                t1[:], ps[:], b_sb[:],
                                op=mybir.AluOpType.add)
                            nc.vector.tensor_scalar(
                                osl, t1[:], 0.0, None,
                                op0=mybir.AluOpType.max)
                        else:
                            nc.vector.tensor_tensor(
                                osl, ps[:], b_sb[:],
                                op=mybir.AluOpType.add)
                    w0 = bb * WPT
                    if layer1:
                        # write into agin chunks, splitting at boundaries
                        i = 0
                        while i < nwin:
                            wv = w0 + i
                            kq = next(kk for kk in range(NK)
                                      if WSTART[kk] <= wv
                                      < WSTART[kk] + WK[kk])
                            nblk = min(nwin - i,
                                       WSTART[kq] + WK[kq] - wv)
                            r0 = 128 * (wv - WSTART[kq])
                            nc.sync.dma_start(
                                agin[kq].ap()[r0:r0 + 128 * nblk, :]
                                .rearrange("(t p) f -> p t f", p=128),
                                ob[:, 128 * i:128 * (i + nblk)].rearrange(
                                    "p (t f) -> p t f", f=128))
                            i += nblk
                    else:
                        rows = min(S, 128 * (w0 + nwin)) - 128 * w0
                        nfull = rows // 128
                        if nfull:
                            nc.sync.dma_start(
                                out.ap()[128 * w0:128 * (w0 + nfull), :]
                                .rearrange("(t p) f -> p t f", p=128),
                                ob[:, :128 * nfull].rearrange(
                                    "p (t f) -> p t f", f=128))
                        rem = rows - 128 * nfull
                        if rem:
                            nc.sync.dma_start(
                                out.ap()[128 * (w0 + nfull):
                                         128 * (w0 + nfull) + rem, :],
                                ob[0:rem, 128 * nfull:128 * (nfull + 1)])

            for r in range(repeat):
                if "B" in phases:
                    with nc.named_scope("phB"):
                        aggregate(layer1=True)
                if ag:
                    with nc.named_scope("phAG"):
                        for k in range(NK):
                            nc.gpsimd.collective_compute(
                                "AllGather", mybir.AluOpType.bypass,
                                replica_groups=[list(range(N_CORES))],
                                ins=[agin[k][:].opt()],
                                outs=[agout[k][:].opt()])
                if "D" in phases:
                    with nc.named_scope("phD"):
                        aggregate(layer1=False)

    # Align each gather's SWDGE queue with its Tile-assigned DMASW sem lane
    # (queue = lane % 4): the scheduler may reorder gathers after my
    # round-robin queue choice, and a lane semaphore must not be updated
    # from two different queues while in flight.
    from concourse.tile_sem_assignment import PROC_NAME_TO_IDX
    lane0 = PROC_NAME_TO_IDX["DMASW0"]
    for blk in nc.m.functions[0].blocks:
        for inst in blk.instructions:
            if type(inst).__name__ == "InstDMAGatherAnt":
                inst.queue_num = (inst.bass_scheduled_proc - lane0) % 4
    nc.compile()
    return nc


def _make_in_maps(x, W1, b1, W2, b2, sch):
    xv = np.asarray(x, np.float32)
    xqs = {}
    for k in range(NK):
        t = np.zeros((8 * ROWS_K[k], D), np.float32)
        for c in range(N_CORES):
            l0, l1 = RSTART[k], min(RSTART[k] + ROWS_K[k], S)
            t[c * ROWS_K[k]:c * ROWS_K[k] + (l1 - l0)] = \
                xv[c * S + l0:c * S + l1]
        xqs[f"xq{k}"] = t
    iota = np.broadcast_to(np.arange(128, dtype=np.float32),
                           (128, 128)).astype(ml_dtypes.bfloat16)
    common = {
        **xqs,
        "W1b": np.asarray(W1, np.float32).astype(ml_dtypes.bfloat16),
        "W2b": np.asarray(W2, np.float32).astype(ml_dtypes.bfloat16),
        "b1b": np.broadcast_to(np.asarray(b1, np.float32), (128, 128)).copy(),
        "b2b": np.broadcast_to(np.asarray(b2, np.float32), (128, 128)).copy(),
        "iotab": np.ascontiguousarray(iota),
    }
    in_maps = []
    for c in range(N_CORES):
        in_maps.append({
            **common,
            "gx": _wrap_idx(sch["gidx"][c]),
            "dSe": sch["dSem"][c],
            "dd": sch["dd"][c],
        })
    return in_maps


def kernel(x, edge_index, W1, b1, W2, b2):
    sch = _prep(edge_index)
    nc = _build(sch, repeat=int(os.environ.get("KERNEL_REPEAT", "1")))
    in_maps = _make_in_maps(x, W1, b1, W2, b2, sch)
    res = run_bass_kernel_spmd(nc, in_maps, core_ids=list(range(N_CORES)))
    return np.concatenate([res.results[c]["out"] for c in range(N_CORES)], 0)
